# revision 1
# baseline (speedup 1.0000x reference)
"""TRN2 Bass kernel for nn_ChartOperator (sparse_attention).

Math (B=4, N=4096, PD=1024, D=16, S=64, ALL=1024):
  P = x @ W_r + b_r
  L = P[..., :ALL].reshape(n, D, S); R = P[..., ALL:].reshape(n, D, S)
  w = softmax_a(L)
  Q[n, d, s] = sum_{a<d} w[n,a,s] * R[n+a+1, d-1-a, s]
  (last D rows of each batch: Q[t+d>=16] zeroed)
  out = Q.reshape(n, ALL) @ W_w + b_w

Sharding: 8 cores, data-parallel over flattened (B*N) rows, 2048 rows/core
plus a 128-row forward halo (zero-padded at batch boundaries; the affected
outputs are exactly the masked ones).

Per-core pipeline (bf16 matmuls, fp32 PSUM):
  1. Reader computes P TRANSPOSED: psum[(d2,s64), n512] = W_r_slab.T @ xT
     (bias via K=1 matmul with b_r slab as stationary, ones moving).
  2. ACT exp/copy writes the banded-MAC layout directly:
     e chain  et_all[(g2,s64), a16, n1024]   (g: row-blocks 0-7 / 8-15)
     r chain  rt_all[(g2,s64), c16, n1152]   (blocks 0-8 / 8-16 incl halo)
  3. DVE: Z = reduce over a; reciprocal; normalize et_all in place.
  4. DVE banded products p[s, c, n] = w[s,a,n]*r[s,c,n+a+1] (single op per a)
  5. PE identity-matmuls accumulate products into PSUM Q[(g,s), d, n]
     with shrinking d-windows (d = a+c+1)
  6. ACT strided copies Q -> qt[(dsub2,s64), k8, n128] bf16 (writer lhsT)
  7. writer matmuls out[n128, 1024] = qt.T @ W_w + b_w -> DMA out
"""
import numpy as np
import ml_dtypes
from contextlib import ExitStack

import concourse.bass as bass
import concourse.tile as tile
from concourse import bacc, mybir
from concourse import bass_utils

BF16 = mybir.dt.bfloat16
F32 = mybir.dt.float32
bfnp = ml_dtypes.bfloat16

B, N, PD = 4, 4096, 1024
D, S = 16, 64
ALL = D * S
ROWS_PER_CORE = 2048
NROWS = 2176                   # + 128-row halo
NCP = 8

_cache = {}


def _build(debug=False):
    nc = bacc.Bacc("TRN2", target_bir_lowering=False, debug=False, num_devices=8)

    xT_d = nc.dram_tensor("xT", [8, 128, NROWS], BF16, kind="ExternalInput").ap()
    wr_d = nc.dram_tensor("wr", [8, 128, 2048], BF16, kind="ExternalInput").ap()
    ww_d = nc.dram_tensor("ww", [8, 128, 1024], BF16, kind="ExternalInput").ap()
    br_d = nc.dram_tensor("br", [128, 16], F32, kind="ExternalInput").ap()
    ident_d = nc.dram_tensor("ident", [128, 128], BF16, kind="ExternalInput").ap()
    qmask_d = nc.dram_tensor("qmask", [128, 8, 128], BF16, kind="ExternalInput").ap()
    out_d = nc.dram_tensor("out", [16, 128, 1024], F32, kind="ExternalOutput").ap()

    with tile.TileContext(nc) as tc, ExitStack() as ctx:
        cpool = ctx.enter_context(tc.tile_pool(name="cpool", bufs=1))
        ps512 = ctx.enter_context(tc.tile_pool(name="ps512", bufs=2, space="PSUM"))
        wps = ctx.enter_context(tc.tile_pool(name="wps", bufs=2, space="PSUM"))
        ztp = ctx.enter_context(tc.tile_pool(name="ztp", bufs=1))
        rzp = ctx.enter_context(tc.tile_pool(name="rzp", bufs=1))
        prodp = ctx.enter_context(tc.tile_pool(name="prodp", bufs=4))
        macp = ctx.enter_context(tc.tile_pool(name="macp", bufs=1, space="PSUM"))
        qtp = ctx.enter_context(tc.tile_pool(name="qtp", bufs=3))
        osbp = ctx.enter_context(tc.tile_pool(name="osbp", bufs=2))

        # --- persistent constants / big buffers
        xk = [cpool.tile([128, NROWS], BF16, name=f"xk{i}", tag=f"xk{i}")
              for i in range(8)]
        for ks in range(8):
            nc.gpsimd.dma_start(xk[ks][:], xT_d[ks])
        wr_sb = cpool.tile([128, 8, 2048], BF16)
        nc.gpsimd.dma_start(wr_sb[:], wr_d[:].rearrange("k p c -> p k c"))
        ww_sb = cpool.tile([128, 8, 1024], BF16)
        nc.gpsimd.dma_start(ww_sb[:], ww_d[:].rearrange("k p c -> p k c"))
        br_sb = cpool.tile([128, 16], F32)
        nc.gpsimd.dma_start(br_sb[:], br_d[:])
        ident = cpool.tile([128, 128], BF16)
        nc.gpsimd.dma_start(ident[:], ident_d[:])
        qmask = cpool.tile([128, 8, 128], BF16)
        nc.gpsimd.dma_start(qmask[:], qmask_d[:])
        et_all = cpool.tile([128, 16, 1024], BF16)   # [(g2,s64), a, n-chain]
        rt_all = cpool.tile([128, 16, 1152], BF16)   # [(g2,s64), c, n-chain]

        # ---------------- Loop 1: transposed reader + layout writes
        # jj: 4 supertiles of 512 rows + 1 halo tile of 128 rows
        for jj in (0, 2, 1, 3, 4):
            nwin = 128 if jj == 4 else 512
            n0 = jj * 512
            g = 0 if jj < 2 else 1
            for u in range(16):               # col slabs: 0-7 = L, 8-15 = R
                is_l = u < 8
                if is_l and jj == 4:
                    continue                  # halo rows: R only
                ps = ps512.tile([128, 512], F32, tag="ps512", name="ps")
                for ks in range(8):
                    nc.tensor.matmul(ps[:, :nwin], wr_sb[:, ks, 128 * u:128 * (u + 1)],
                                     xk[ks][:, n0:n0 + nwin],
                                     start=(ks == 0), stop=(ks == 7))
                for dsub in range(2):
                    src = ps[64 * dsub:64 * dsub + 64, :nwin]
                    bias = br_sb[64 * dsub:64 * dsub + 64, u:u + 1]
                    AF = mybir.ActivationFunctionType
                    if is_l:
                        a = 2 * u + dsub
                        dst = et_all[64 * g:64 * g + 64, a,
                                     n0 - 1024 * g:n0 - 1024 * g + nwin]
                        nc.scalar.activation(dst, src, AF.Exp, bias=bias)
                    else:
                        c = 2 * (u - 8) + dsub
                        # g0 chain: blocks 0..8 ; g1 chain: blocks 8..16
                        if jj < 2:
                            nc.scalar.activation(rt_all[0:64, c, n0:n0 + 512], src,
                                                 AF.Identity, bias=bias)
                        elif jj == 2:   # blocks 8-11: both chains
                            nc.scalar.activation(rt_all[0:64, c, 1024:1152],
                                                 ps[64 * dsub:64 * dsub + 64, 0:128],
                                                 AF.Identity, bias=bias)
                            nc.scalar.activation(rt_all[64:128, c, 0:512], src,
                                                 AF.Identity, bias=bias)
                        elif jj == 3:
                            nc.scalar.activation(rt_all[64:128, c, 512:1024], src,
                                                 AF.Identity, bias=bias)
                        else:           # halo block 16
                            nc.scalar.activation(rt_all[64:128, c, 1024:1152], src,
                                                 AF.Identity, bias=bias)

            if jj in (2, 3):
                # normalize chain-window: w0 = supertiles {0,2}, w1 = {1,3}
                # (loop order 0,2,1,3 makes w0 ready after the 2nd supertile)
                lo = (jj - 2) * 512
                zt = ztp.tile([128, 512], F32, tag="ztp", name="zt")
                e_na = et_all[:, :, lo:lo + 512].rearrange("p a n -> p n a")
                nc.vector.tensor_reduce(zt[:], e_na, axis=mybir.AxisListType.X,
                                        op=mybir.AluOpType.add)
                rz = rzp.tile([128, 512], F32, tag="rzp", name="rz")
                nc.vector.reciprocal(rz[:], zt[:])
                rz3 = rz[:].rearrange("p (o n) -> p o n", o=1).to_broadcast((128, 16, 512))
                ew = et_all[:, :, lo:lo + 512]
                nc.vector.tensor_mul(ew, ew, rz3)

        # ---------------- Loop 2+3: banded MAC + writer per chunklet-pair
        for cp in range(NCP):
            mp = macp.tile([128, 16, 128], F32, tag="macp", name="mp")
            nc.vector.memset(mp[:, 0, :], 0.0)
            n0 = 128 * cp
            for a in range(15):
                cnt = 15 - a
                p = prodp.tile([128, 15, 128], BF16, tag="prodp", name="p")
                eb = et_all[:, a:a + 1, n0:n0 + 128].to_broadcast((128, cnt, 128))
                # odd (a+1)-shifts run at DVE 1x (4B misalignment); split
                # those across DVE and GpSimd to balance the two engines
                if a % 2 == 0 and cnt >= 2:
                    cs = max(1, (3 * cnt) // 5)
                    nc.vector.tensor_mul(p[:, 0:cs, :], eb[:, 0:cs, :],
                                         rt_all[:, 0:cs, n0 + a + 1:n0 + a + 129])
                    nc.gpsimd.tensor_mul(p[:, cs:cnt, :], eb[:, 0:cnt - cs, :],
                                         rt_all[:, cs:cnt, n0 + a + 1:n0 + a + 129])
                else:
                    nc.vector.tensor_mul(p[:, 0:cnt, :], eb,
                                         rt_all[:, 0:cnt, n0 + a + 1:n0 + a + 129])
                for b in range(4):
                    d_lo = max(a + 1, 4 * b)
                    d_hi = 4 * b + 4
                    if d_lo >= d_hi:
                        continue
                    last_a = min(14, 4 * b + 2)
                    nc.tensor.matmul(mp[:, d_lo:d_hi, :], ident[:],
                                     p[:, d_lo - a - 1:d_hi - a - 1, :],
                                     start=(a == 0), stop=(a == last_a))

            for g in range(2):
                cb = 8 * g + cp
                qt = qtp.tile([128, 8, 128], BF16, tag="qtp", name="qt")
                for dsub in range(2):
                    csrc = mp[64 * g:64 * g + 64, dsub::2, :]
                    cdst = qt[64 * dsub:64 * dsub + 64, :, :]
                    if g == 0:
                        nc.scalar.copy(cdst, csrc)
                    else:
                        nc.vector.tensor_copy(cdst, csrc)
                if cb == 15:
                    nc.vector.tensor_mul(qt[:], qt[:], qmask[:])
                osb = osbp.tile([128, 1024], F32, tag="osbp", name="osb")
                for h in range(2):
                    wp = wps.tile([128, 512], F32, tag="wps", name="wp")
                    for k in range(8):
                        nc.tensor.matmul(wp[:], qt[:, k, :],
                                         ww_sb[:, k, h * 512:(h + 1) * 512],
                                         start=(k == 0), stop=(k == 7))
                    nc.vector.tensor_copy(osb[:, h * 512:(h + 1) * 512], wp[:])
                nc.gpsimd.dma_start(out_d[cb], osb[:])

    nc.compile()
    return nc


def _host_prep(x, W_r, b_r, W_w, b_w):
    """Build the 8 per-core input maps."""
    xf = np.asarray(x, np.float32).reshape(B * N, PD)
    wr = np.asarray(W_r, np.float32).astype(bfnp)
    ww = np.asarray(W_w, np.float32).astype(bfnp)
    br = np.ascontiguousarray(
        np.asarray(b_r, np.float32).reshape(16, 128).T)
    wr_t = np.ascontiguousarray(wr.reshape(8, 128, 2048))
    ww_t = np.ascontiguousarray(ww.reshape(8, 128, 1024))
    ident = np.eye(128, dtype=np.float32).astype(bfnp)

    in_maps = []
    for c in range(8):
        lo = c * ROWS_PER_CORE
        chunk = np.zeros((NROWS, PD), np.float32)
        chunk[:ROWS_PER_CORE] = xf[lo:lo + ROWS_PER_CORE]
        if c % 2 == 0:
            chunk[ROWS_PER_CORE:] = xf[lo + ROWS_PER_CORE: lo + NROWS]
        # xT[ks, k, n] = chunk[n, 128*ks + k]
        xt = np.ascontiguousarray(
            chunk.astype(bfnp).reshape(NROWS, 8, 128).transpose(1, 2, 0))
        qmask = np.ones((128, 8, 128), np.float32)
        if c % 2 == 1:
            dsub = (np.arange(128)[:, None, None] // 64)
            k = np.arange(8)[None, :, None]
            n = np.arange(128)[None, None, :]
            bad = (n >= 112) & ((n - 112 + 2 * k + dsub) >= 16)
            qmask[np.broadcast_to(bad, (128, 8, 128))] = 0.0
        in_maps.append({
            "xT": xt,
            "wr": wr_t, "ww": ww_t, "br": br,
            "ident": ident, "qmask": qmask.astype(bfnp),
        })
    return in_maps


def kernel(x, W_r, b_r, W_w, b_w):
    if "nc" not in _cache:
        _cache["nc"] = _build()
    nc = _cache["nc"]
    in_maps = _host_prep(x, W_r, b_r, W_w, b_w)
    res = bass_utils.run_bass_kernel_spmd(nc, in_maps, core_ids=list(range(8)))
    out = np.concatenate([r["out"].reshape(ROWS_PER_CORE, ALL)
                          for r in res.results], axis=0)
    out = out.reshape(B, N, ALL).astype(np.float32)
    out += np.asarray(b_w, np.float32).reshape(1, 1, ALL)
    return np.ascontiguousarray(out)



# revision 6
# speedup vs baseline: 1.0923x; 1.0923x over previous
"""TRN2 Bass kernel for nn_ChartOperator (sparse_attention).

Math (B=4, N=4096, PD=1024, D=16, S=64, ALL=1024):
  P = x @ W_r + b_r
  L = P[..., :ALL].reshape(n, D, S); R = P[..., ALL:].reshape(n, D, S)
  w = softmax_a(L)
  Q[n, d, s] = sum_{a<d} w[n,a,s] * R[n+a+1, d-1-a, s]
  (last D rows of each batch: Q[t+d>=16] zeroed)
  out = Q.reshape(n, ALL) @ W_w + b_w

Sharding: 8 cores, data-parallel over flattened (B*N) rows, 2048 rows/core
plus a 16-row forward halo (zero-padded at batch boundaries; the affected
outputs are exactly the masked ones).

Per-core pipeline (bf16 matmuls, fp32 PSUM), interleaved for engine overlap:
  reader supertiles jj (512 rows each, 0/2 then 1 then 3 then halo) compute
  P transposed into PSUM; ACT exp/copy writes banded-MAC chains
     e chain  et_all[(g2,s64), a16, n1024]   (g: row-blocks 0-7 / 8-15)
     r chain  rt_all[(g2,s64), c15, n1040]   (+ rt_sh, shifted 1 col, so
     every DVE band product is 4B-aligned -> 2x mode)
  softmax Z via chained bf16 adds + reciprocal_approx_fast -> rzb (bf16)
  per chunk cp (128 rows): ebn = e-slice * rz; band products on DVE/GpSimd;
  PE identity-matmuls accumulate into PSUM Q[(g,s), d, n] per 4-d bank;
  per-bank ACT/DVE copies Q -> qt bf16 as soon as each bank closes;
  writer matmuls out[n128, 1024] = qt.T @ W_w -> bf16 DMA out (host adds b_w)
Chunk work for rows covered by supertiles 0/2 is emitted between reader
supertiles so PE / ACT / DVE / GpSimd queues always hold ready work.
"""
import numpy as np
import ml_dtypes
from contextlib import ExitStack

import concourse.bass as bass
import concourse.tile as tile
from concourse import bacc, mybir
from concourse import bass_utils

BF16 = mybir.dt.bfloat16
F32 = mybir.dt.float32
bfnp = ml_dtypes.bfloat16

B, N, PD = 4, 4096, 1024
D, S = 16, 64
ALL = D * S
ROWS_PER_CORE = 2048
HALO = 16
NCP = 8                         # 128-row chunk pairs per chain window

_cache = {}

AF = mybir.ActivationFunctionType
LAST_A = [min(14, 4 * b + 2) for b in range(4)]   # bank b closes after this a


def _build(debug=False):
    nc = bacc.Bacc("TRN2", target_bir_lowering=False, debug=False, num_devices=8)

    # x transposed, per 512-row window: xw[w, ks, k, n]
    xw_d = nc.dram_tensor("xw", [4, 8, 128, 512], BF16, kind="ExternalInput").ap()
    xh_d = nc.dram_tensor("xh", [8, 128, HALO], BF16, kind="ExternalInput").ap()
    wr_d = nc.dram_tensor("wr", [8, 128, 2048], BF16, kind="ExternalInput").ap()
    ww_d = nc.dram_tensor("ww", [8, 128, 1024], BF16, kind="ExternalInput").ap()
    br_d = nc.dram_tensor("br", [128, 16], F32, kind="ExternalInput").ap()
    ident_d = nc.dram_tensor("ident", [128, 128], BF16, kind="ExternalInput").ap()
    qmask_d = nc.dram_tensor("qmask", [128, 8, 128], BF16, kind="ExternalInput").ap()
    out_d = nc.dram_tensor("out", [16, 128, 1024], BF16, kind="ExternalOutput").ap()

    with tile.TileContext(nc) as tc, ExitStack() as ctx:
        cpool = ctx.enter_context(tc.tile_pool(name="cpool", bufs=1))
        xkp = ctx.enter_context(tc.tile_pool(name="xkp", bufs=2))
        ps512 = ctx.enter_context(tc.tile_pool(name="ps512", bufs=2, space="PSUM"))
        wps = ctx.enter_context(tc.tile_pool(name="wps", bufs=2, space="PSUM"))
        macp = ctx.enter_context(tc.tile_pool(name="macp", bufs=1, space="PSUM"))
        zpool = ctx.enter_context(tc.tile_pool(name="zpool", bufs=1))
        ebnp = ctx.enter_context(tc.tile_pool(name="ebnp", bufs=2))
        prodp = ctx.enter_context(tc.tile_pool(name="prodp", bufs=3))
        prodg = ctx.enter_context(tc.tile_pool(name="prodg", bufs=4))
        qtp = ctx.enter_context(tc.tile_pool(name="qtp", bufs=3))
        osbp = ctx.enter_context(tc.tile_pool(name="osbp", bufs=2))

        # ---- constants / chains (persistent)
        wr_sb = cpool.tile([128, 8, 2048], BF16)
        br_sb = cpool.tile([128, 16], F32)
        ww_sb = cpool.tile([128, 8, 1024], BF16)
        ident = cpool.tile([128, 128], BF16)
        qmask = cpool.tile([128, 8, 128], BF16)
        xh = cpool.tile([128, 8, HALO], BF16)
        et_all = cpool.tile([128, 16, 1024], BF16)   # [(g2,s64), a, n-chain]
        rt_all = cpool.tile([128, 15, 1040], BF16)   # [(g2,s64), c, n-chain]
        rt_sh = cpool.tile([128, 15, 1040], BF16)    # rt_sh[n] = rt_all[n+1]
        zb = cpool.tile([128, 4, 512], BF16)
        zt = cpool.tile([128, 512], F32)
        rz = cpool.tile([128, 512], F32)
        rzb = cpool.tile([128, 1024], BF16)          # 1/Z bf16, both windows

        # ---- DMA: ordered so the first reader matmul unblocks early
        nc.gpsimd.dma_start(br_sb[:], br_d[:])
        xk = {}
        xk[0] = xkp.tile([128, 8, 512], BF16, tag="xk", name="xk0")
        nc.gpsimd.dma_start(xk[0][:], xw_d[0].rearrange("k p c -> p k c"))
        for c4 in range(4):    # wr in 4 column chunks of 4 u-slabs
            nc.gpsimd.dma_start(wr_sb[:, :, 512 * c4:512 * (c4 + 1)],
                                wr_d[:, :, 512 * c4:512 * (c4 + 1)]
                                .rearrange("k p c -> p k c"))
        xk[2] = xkp.tile([128, 8, 512], BF16, tag="xk", name="xk2")
        nc.gpsimd.dma_start(xk[2][:], xw_d[2].rearrange("k p c -> p k c"))
        nc.gpsimd.dma_start(ident[:], ident_d[:])
        nc.gpsimd.dma_start(ww_sb[:], ww_d[:].rearrange("k p c -> p k c"))
        nc.gpsimd.dma_start(qmask[:], qmask_d[:])
        nc.gpsimd.dma_start(xh[:], xh_d[:].rearrange("k p c -> p k c"))

        # ---------------- reader supertile ----------------
        def reader(jj):
            """supertile jj: rows [512*jj, 512*jj+512). g/chain-col mapping:
            jj 0,1 -> g0 cols 512*jj ; jj 2,3 -> g1 cols 512*(jj-2).
            Extra tails: jj2 R rows 1024:1040 -> g0 cols 1024:1040;
            jj4 = halo (16 rows) -> g1 cols 1024:1040, R only."""
            if jj == 4:
                for u in range(8, 16):
                    ps = ps512.tile([128, 512], F32, tag="ps512", name="ps")
                    for ks in range(8):
                        nc.tensor.matmul(ps[:, :HALO], wr_sb[:, ks, 128 * u:128 * (u + 1)],
                                         xh[:, ks, :], start=(ks == 0), stop=(ks == 7))
                    for dsub in range(2):
                        c = 2 * (u - 8) + dsub
                        if c == 15:
                            continue
                        nc.scalar.activation(rt_all[64:128, c, 1024:1024 + HALO],
                                             ps[64 * dsub:64 * dsub + 64, :HALO],
                                             AF.Identity,
                                             bias=br_sb[64 * dsub:64 * dsub + 64, u:u + 1])
                return
            n0 = 512 * jj
            g = 0 if jj < 2 else 1
            lo = 512 * (jj % 2)             # chain col base
            p0 = 64 * g
            for u in range(16):
                is_l = u < 8
                ps = ps512.tile([128, 512], F32, tag="ps512", name="ps")
                for ks in range(8):
                    nc.tensor.matmul(ps[:], wr_sb[:, ks, 128 * u:128 * (u + 1)],
                                     xk[jj][:, ks, :], start=(ks == 0), stop=(ks == 7))
                for dsub in range(2):
                    src = ps[64 * dsub:64 * dsub + 64, :]
                    bias = br_sb[64 * dsub:64 * dsub + 64, u:u + 1]
                    if is_l:
                        a = 2 * u + dsub
                        nc.scalar.activation(et_all[p0:p0 + 64, a, lo:lo + 512],
                                             src, AF.Exp, bias=bias)
                    else:
                        c = 2 * (u - 8) + dsub
                        if c == 15:
                            continue
                        nc.scalar.activation(rt_all[p0:p0 + 64, c, lo:lo + 512],
                                             src, AF.Identity, bias=bias)
                        if jj == 2:   # g0 chain tail rows 1024:1040
                            nc.scalar.activation(rt_all[0:64, c, 1024:1024 + HALO],
                                                 ps[64 * dsub:64 * dsub + 64, :HALO],
                                                 AF.Identity, bias=bias)

        # ---------------- rt_sh shifted-copy regions ----------------
        def rtsh(g, c0, c1):
            """rt_sh[g, :, c0:c1] = rt_all[g, :, c0+1:c1+1] (DVE copy)."""
            p0 = 64 * g
            nc.vector.tensor_copy(rt_sh[p0:p0 + 64, :, c0:c1],
                                  rt_all[p0:p0 + 64, :, c0 + 1:c1 + 1])

        # ---------------- softmax 1/Z for window w (chain cols lo:lo+512) ----
        def softz(w):
            lo = 512 * w
            e = et_all[:, :, lo:lo + 512]
            nc.vector.tensor_add(zb[:], e[:, 0:4, :], e[:, 4:8, :])
            nc.vector.tensor_add(zb[:], zb[:], e[:, 8:12, :])
            nc.vector.tensor_add(zb[:], zb[:], e[:, 12:16, :])
            nc.vector.tensor_add(zb[:, 0:2, :], zb[:, 0:2, :], zb[:, 2:4, :])
            nc.vector.tensor_add(zt[:], zb[:, 0, :], zb[:, 1, :])
            nc.vector.reciprocal_approx_fast(rz[:], zt[:])
            nc.vector.tensor_copy(rzb[:, lo:lo + 512], rz[:])

        # ---------------- one 128-row chunk pair ----------------
        GP_A = (10, 11, 12, 13, 14)        # band products routed to GpSimd
        def chunk(cp):
            n0 = 128 * cp
            ebn = ebnp.tile([128, 15, 128], BF16, tag="ebn", name="ebn")
            rzs = rzb[:, n0:n0 + 128].rearrange("p (o n) -> p o n", o=1) \
                .to_broadcast((128, 15, 128))
            nc.vector.tensor_mul(ebn[:], et_all[:, 0:15, n0:n0 + 128], rzs)
            mp = macp.tile([128, 16, 128], F32, tag="macp", name="mp")
            nc.vector.memset(mp[:, 0, :], 0.0)
            qt = {g: qtp.tile([128, 8, 128], BF16, tag="qtp", name=f"qt{g}")
                  for g in range(2)}

            def qt_copy(b, ks):
                """evacuate bank b, qt slab(s) ks, as soon as the bank closes"""
                for g in range(2):
                    masked = (cp == 7 and g == 1)
                    for dsub in range(2):
                        for k in ks:
                            csrc = mp[64 * g:64 * g + 64, 2 * k + dsub, :]
                            cdst = qt[g][64 * dsub:64 * dsub + 64, k, :]
                            if masked:
                                qm = qmask[64 * dsub:64 * dsub + 64, k, :]
                                nc.vector.tensor_mul(cdst, csrc, qm)
                            else:
                                nc.scalar.copy(cdst, csrc)

            for a in range(15):
                cnt = 15 - a
                if a in GP_A:
                    p = prodg.tile([128, 5, 128], BF16, tag="prodg", name="pg")
                    eng = nc.gpsimd
                else:
                    p = prodp.tile([128, 15, 128], BF16, tag="prodp", name="p")
                    eng = nc.vector
                eb = ebn[:, a:a + 1, :].to_broadcast((128, cnt, 128))
                if a % 2 == 0:
                    rsrc = rt_sh[:, 0:cnt, n0 + a:n0 + a + 128]
                else:
                    rsrc = rt_all[:, 0:cnt, n0 + a + 1:n0 + a + 129]
                eng.tensor_mul(p[:, 0:cnt, :], eb, rsrc)
                for b in range(4):
                    d_lo = max(a + 1, 4 * b)
                    d_hi = 4 * b + 4
                    if d_lo >= d_hi:
                        continue
                    nc.tensor.matmul(mp[:, d_lo:d_hi, :], ident[:],
                                     p[:, d_lo - a - 1:d_hi - a - 1, :],
                                     start=(a == 0), stop=(a == LAST_A[b]))
                # per-bank evacuation right after the closing matmul
                if a == 2:
                    qt_copy(0, (0, 1))
                elif a == 6:
                    qt_copy(1, (2, 3))
                elif a == 10:
                    qt_copy(2, (4, 5))
                elif a == 14:
                    qt_copy(3, (6, 7))

            for g in range(2):
                cb = 8 * g + cp
                osb = osbp.tile([128, 1024], BF16, tag="osbp", name="osb")
                for h in range(2):
                    wp = wps.tile([128, 512], F32, tag="wps", name="wp")
                    for k in range(8):
                        nc.tensor.matmul(wp[:], qt[g][:, k, :],
                                         ww_sb[:, k, h * 512:(h + 1) * 512],
                                         start=(k == 0), stop=(k == 7))
                    nc.scalar.copy(osb[:, h * 512:(h + 1) * 512], wp[:])
                nc.gpsimd.dma_start(out_d[cb], osb[:])

        # ---------------- interleaved program ----------------
        reader(0)
        rtsh(0, 0, 511)                      # needs jj0 R
        reader(2)
        # prefetch remaining x windows (WAR on jj0/jj2 matmuls via Tile)
        xk[1] = xkp.tile([128, 8, 512], BF16, tag="xk", name="xk1")
        nc.gpsimd.dma_start(xk[1][:], xw_d[1].rearrange("k p c -> p k c"))
        xk[3] = xkp.tile([128, 8, 512], BF16, tag="xk", name="xk3")
        nc.gpsimd.dma_start(xk[3][:], xw_d[3].rearrange("k p c -> p k c"))
        softz(0)                             # needs jj0+jj2 L
        rtsh(1, 0, 160)                      # needs jj2 R -> unblocks cp0
        rtsh(1, 160, 511)
        rtsh(0, 1023, 1039)                  # jj2 g0 tail
        chunk(0)
        chunk(1)
        chunk(2)
        reader(1)
        rtsh(0, 511, 1023)                   # needs jj1 R
        reader(3)
        rtsh(1, 511, 544)                    # needs jj3 R -> unblocks cp3
        softz(1)                             # needs jj1+jj3 L
        chunk(3)
        reader(4)
        rtsh(1, 544, 1023)
        rtsh(1, 1023, 1039)                  # needs jj4 halo R
        chunk(4)
        chunk(5)
        chunk(6)
        chunk(7)

    nc.compile()
    return nc


def _host_prep(x, W_r, b_r, W_w, b_w):
    """Build the 8 per-core input maps."""
    xf = np.asarray(x, np.float32).reshape(B * N, PD)
    wr = np.asarray(W_r, np.float32).astype(bfnp)
    ww = np.asarray(W_w, np.float32).astype(bfnp)
    br = np.ascontiguousarray(
        np.asarray(b_r, np.float32).reshape(16, 128).T)
    wr_t = np.ascontiguousarray(wr.reshape(8, 128, 2048))
    ww_t = np.ascontiguousarray(ww.reshape(8, 128, 1024))
    ident = np.eye(128, dtype=np.float32).astype(bfnp)

    in_maps = []
    for c in range(8):
        lo = c * ROWS_PER_CORE
        chunk = np.zeros((ROWS_PER_CORE + HALO, PD), np.float32)
        chunk[:ROWS_PER_CORE] = xf[lo:lo + ROWS_PER_CORE]
        if c % 2 == 0:
            chunk[ROWS_PER_CORE:] = xf[lo + ROWS_PER_CORE: lo + ROWS_PER_CORE + HALO]
        cb = chunk.astype(bfnp)
        # xw[w, ks, k, n] = chunk[512*w + n, 128*ks + k]
        xw = np.ascontiguousarray(
            cb[:ROWS_PER_CORE].reshape(4, 512, 8, 128).transpose(0, 2, 3, 1))
        xh = np.ascontiguousarray(
            cb[ROWS_PER_CORE:].reshape(HALO, 8, 128).transpose(1, 2, 0))
        qmask = np.ones((128, 8, 128), np.float32)
        if c % 2 == 1:
            dsub = (np.arange(128)[:, None, None] // 64)
            k = np.arange(8)[None, :, None]
            n = np.arange(128)[None, None, :]
            bad = (n >= 112) & ((n - 112 + 2 * k + dsub) >= 16)
            qmask[np.broadcast_to(bad, (128, 8, 128))] = 0.0
        in_maps.append({
            "xw": xw, "xh": xh,
            "wr": wr_t, "ww": ww_t, "br": br,
            "ident": ident, "qmask": qmask.astype(bfnp),
        })
    return in_maps


def kernel(x, W_r, b_r, W_w, b_w):
    if "nc" not in _cache:
        _cache["nc"] = _build()
    nc = _cache["nc"]
    in_maps = _host_prep(x, W_r, b_r, W_w, b_w)
    res = bass_utils.run_bass_kernel_spmd(nc, in_maps, core_ids=list(range(8)))
    out = np.concatenate([np.asarray(r["out"], np.float32)
                          .reshape(ROWS_PER_CORE, ALL)
                          for r in res.results], axis=0)
    out = out.reshape(B, N, ALL)
    out += np.asarray(b_w, np.float32).reshape(1, 1, ALL)
    return np.ascontiguousarray(out)


# revision 13
# speedup vs baseline: 1.1553x; 1.0576x over previous
"""TRN2 Bass kernel for nn_ChartOperator (sparse_attention).

Math (B=4, N=4096, PD=1024, D=16, S=64, ALL=1024):
  P = x @ W_r + b_r
  L = P[..., :ALL].reshape(n, D, S); R = P[..., ALL:].reshape(n, D, S)
  w = softmax_a(L)
  Q[n, d, s] = sum_{a<d} w[n,a,s] * R[n+a+1, d-1-a, s]
  (last D rows of each batch: Q[t+d>=16] zeroed)
  out = Q.reshape(n, ALL) @ W_w + b_w

Sharding: 8 cores, data-parallel over flattened (B*N) rows, 2048 rows/core
plus a 16-row forward halo (zero-padded at batch boundaries; the affected
outputs are exactly the masked ones).

Per-core pipeline (bf16 matmuls, fp32 PSUM), interleaved for engine overlap:
  reader supertiles jj (512 rows each, 0/2 then 1 then 3 then halo) compute
  P transposed into PSUM; ACT exp/copy writes banded-MAC chains
     e chain  et_all[(g2,s64), a16, n1024]   (g: row-blocks 0-7 / 8-15)
     r chain  rt_all[(g2,s64), c15, n1040]   (+ rt_sh, shifted 1 col, so
     every DVE band product is 4B-aligned -> 2x mode)
  softmax Z via chained bf16 adds + reciprocal_approx_fast -> rzb (bf16)
  per chunk cp (128 rows): ebn = e-slice * rz; band products on DVE/GpSimd;
  PE identity-matmuls accumulate into PSUM Q[(g,s), d, n] per 4-d bank;
  per-bank ACT/DVE copies Q -> qt bf16 as soon as each bank closes;
  writer matmuls out[n128, 1024] = qt.T @ W_w -> bf16 DMA out (host adds b_w)
Chunk work for rows covered by supertiles 0/2 is emitted between reader
supertiles so PE / ACT / DVE / GpSimd queues always hold ready work.
"""
import numpy as np
import ml_dtypes
from contextlib import ExitStack

import concourse.bass as bass
import concourse.tile as tile
from concourse import bacc, mybir
from concourse import bass_utils

BF16 = mybir.dt.bfloat16
F32 = mybir.dt.float32
bfnp = ml_dtypes.bfloat16

B, N, PD = 4, 4096, 1024
D, S = 16, 64
ALL = D * S
ROWS_PER_CORE = 2048
HALO = 16
NCP = 8                         # 128-row chunk pairs per chain window

_cache = {}

AF = mybir.ActivationFunctionType
LAST_A = [min(14, 4 * b + 2) for b in range(4)]   # bank b closes after this a


def _build(debug=False):
    nc = bacc.Bacc("TRN2", target_bir_lowering=False, debug=False, num_devices=8)

    # x transposed, per 512-row window: xw[w, ks, k, n]
    xw_d = nc.dram_tensor("xw", [4, 8, 128, 512], BF16, kind="ExternalInput").ap()
    xh_d = nc.dram_tensor("xh", [8, 128, HALO], BF16, kind="ExternalInput").ap()
    wr_d = nc.dram_tensor("wr", [8, 128, 2048], BF16, kind="ExternalInput").ap()
    ww_d = nc.dram_tensor("ww", [8, 128, 1024], BF16, kind="ExternalInput").ap()
    br_d = nc.dram_tensor("br", [128, 16], F32, kind="ExternalInput").ap()
    ident_d = nc.dram_tensor("ident", [128, 128], BF16, kind="ExternalInput").ap()
    qmask_d = nc.dram_tensor("qmask", [128, 8, 128], BF16, kind="ExternalInput").ap()
    out_d = nc.dram_tensor("out", [16, 128, 1024], BF16, kind="ExternalOutput").ap()

    with tile.TileContext(nc) as tc, ExitStack() as ctx:
        cpool = ctx.enter_context(tc.tile_pool(name="cpool", bufs=1))
        xkp = ctx.enter_context(tc.tile_pool(name="xkp", bufs=2))
        ps512 = ctx.enter_context(tc.tile_pool(name="ps512", bufs=2, space="PSUM"))
        wps = ctx.enter_context(tc.tile_pool(name="wps", bufs=2, space="PSUM"))
        macp = ctx.enter_context(tc.tile_pool(name="macp", bufs=1, space="PSUM"))
        zpool = ctx.enter_context(tc.tile_pool(name="zpool", bufs=1))
        prodp = ctx.enter_context(tc.tile_pool(name="prodp", bufs=4))
        prodg = ctx.enter_context(tc.tile_pool(name="prodg", bufs=6))
        qtp = ctx.enter_context(tc.tile_pool(name="qtp", bufs=3))
        osbp = ctx.enter_context(tc.tile_pool(name="osbp", bufs=2))

        # ---- constants / chains (persistent)
        wr_sb = cpool.tile([128, 8, 2048], BF16)
        br_sb = cpool.tile([128, 16], F32)
        ww_sb = cpool.tile([128, 8, 1024], BF16)
        ident = cpool.tile([128, 128], BF16)
        qmask = cpool.tile([128, 8, 128], BF16)
        xh = cpool.tile([128, 8, HALO], BF16)
        et_all = cpool.tile([128, 16, 1024], BF16)   # [(g2,s64), a, n-chain]
        rt_all = cpool.tile([128, 15, 1040], BF16)   # [(g2,s64), c, n-chain]
        rt_sh = cpool.tile([128, 15, 1040], BF16)    # rt_sh[n] = rt_all[n+1]
        zb = cpool.tile([128, 4, 512], BF16)
        zt = cpool.tile([128, 512], F32)
        rz = cpool.tile([128, 512], F32)
        rzb = cpool.tile([128, 512], BF16)           # 1/Z bf16, one window

        # ---- DMA: ordered so the first reader matmul unblocks early
        nc.gpsimd.dma_start(br_sb[:], br_d[:])
        xk = {}
        xk[0] = xkp.tile([128, 8, 512], BF16, tag="xk", name="xk0")
        nc.gpsimd.dma_start(xk[0][:], xw_d[0].rearrange("k p c -> p k c"))
        for c0, c1 in ((0, 128), (128, 512), (512, 1024), (1024, 2048)):
            nc.gpsimd.dma_start(wr_sb[:, :, c0:c1],
                                wr_d[:, :, c0:c1].rearrange("k p c -> p k c"))
        xk[2] = xkp.tile([128, 8, 512], BF16, tag="xk", name="xk2")
        nc.gpsimd.dma_start(xk[2][:], xw_d[2].rearrange("k p c -> p k c"))
        nc.gpsimd.dma_start(ww_sb[:], ww_d[:].rearrange("k p c -> p k c"))
        nc.gpsimd.dma_start(ident[:], ident_d[:])
        nc.gpsimd.dma_start(qmask[:], qmask_d[:])
        nc.gpsimd.dma_start(xh[:], xh_d[:].rearrange("k p c -> p k c"))

        # ---------------- reader supertile ----------------
        def reader(jj):
            """supertile jj: rows [512*jj, 512*jj+512). g/chain-col mapping:
            jj 0,1 -> g0 cols 512*jj ; jj 2,3 -> g1 cols 512*(jj-2).
            Extra tails: jj2 R rows 1024:1040 -> g0 cols 1024:1040;
            jj4 = halo (16 rows) -> g1 cols 1024:1040, R only."""
            if jj == 4:
                for u in range(8, 16):
                    ps = ps512.tile([128, 512], F32, tag="ps512", name="ps")
                    for ks in range(8):
                        nc.tensor.matmul(ps[:, :HALO], wr_sb[:, ks, 128 * u:128 * (u + 1)],
                                         xh[:, ks, :], start=(ks == 0), stop=(ks == 7))
                    for dsub in range(2):
                        c = 2 * (u - 8) + dsub
                        if c == 15:
                            continue
                        nc.scalar.activation(rt_all[64:128, c, 1024:1024 + HALO],
                                             ps[64 * dsub:64 * dsub + 64, :HALO],
                                             AF.Identity,
                                             bias=br_sb[64 * dsub:64 * dsub + 64, u:u + 1])
                return
            n0 = 512 * jj
            g = 0 if jj < 2 else 1
            lo = 512 * (jj % 2)             # chain col base
            p0 = 64 * g
            for u in range(16):
                is_l = u < 8
                ps = ps512.tile([128, 512], F32, tag="ps512", name="ps")
                for ks in range(8):
                    nc.tensor.matmul(ps[:], wr_sb[:, ks, 128 * u:128 * (u + 1)],
                                     xk[jj][:, ks, :], start=(ks == 0), stop=(ks == 7))
                for dsub in range(2):
                    src = ps[64 * dsub:64 * dsub + 64, :]
                    bias = br_sb[64 * dsub:64 * dsub + 64, u:u + 1]
                    if is_l:
                        a = 2 * u + dsub
                        nc.scalar.activation(et_all[p0:p0 + 64, a, lo:lo + 512],
                                             src, AF.Exp, bias=bias)
                    else:
                        c = 2 * (u - 8) + dsub
                        if c == 15:
                            continue
                        nc.scalar.activation(rt_all[p0:p0 + 64, c, lo:lo + 512],
                                             src, AF.Identity, bias=bias)
                        if jj == 2:   # g0 chain tail rows 1024:1040
                            nc.scalar.activation(rt_all[0:64, c, 1024:1024 + HALO],
                                                 ps[64 * dsub:64 * dsub + 64, :HALO],
                                                 AF.Identity, bias=bias)

        # ---------------- rt_sh shifted-copy regions ----------------
        def rtsh(g, c0, c1):
            """rt_sh[g, :, c0:c1] = rt_all[g, :, c0+1:c1+1] (DVE copy)."""
            p0 = 64 * g
            nc.vector.tensor_copy(rt_sh[p0:p0 + 64, :, c0:c1],
                                  rt_all[p0:p0 + 64, :, c0 + 1:c1 + 1])

        # ---------------- softmax 1/Z for window w (chain cols lo:lo+512) ----
        def softz(w):
            lo = 512 * w
            e = et_all[:, :, lo:lo + 512]
            nc.vector.tensor_add(zb[:], e[:, 0:4, :], e[:, 4:8, :])
            nc.vector.tensor_add(zb[:], zb[:], e[:, 8:12, :])
            nc.vector.tensor_add(zb[:], zb[:], e[:, 12:16, :])
            nc.vector.tensor_add(zb[:, 0:2, :], zb[:, 0:2, :], zb[:, 2:4, :])
            nc.vector.tensor_add(zt[:], zb[:, 0, :], zb[:, 1, :])
            nc.vector.reciprocal_approx_fast(rz[:], zt[:])
            nc.vector.tensor_copy(rzb[:], rz[:])

        def etmul(j):
            """normalize et chain cols [128j, 128j+128) in place (a rows 0:15)"""
            rzs = rzb[:, (128 * j) % 512:(128 * j) % 512 + 128] \
                .rearrange("p (o n) -> p o n", o=1).to_broadcast((128, 15, 128))
            ecol = et_all[:, 0:15, 128 * j:128 * j + 128]
            nc.vector.tensor_mul(ecol, ecol, rzs)

        # ---------------- one 128-row chunk pair ----------------
        GP_A = (10, 11, 12, 13, 14)        # band products routed to GpSimd
        def chunk(cp):
            n0 = 128 * cp
            mp = macp.tile([128, 16, 128], F32, tag="macp", name="mp")
            nc.vector.memset(mp[:, 0, :], 0.0)
            qt = {g: qtp.tile([128, 8, 128], BF16, tag="qtp", name=f"qt{g}")
                  for g in range(2)}

            def qt_copy(b):
                """evacuate bank b (qt slabs 2b, 2b+1) once the bank closes.
                g0 -> ACT, g1 -> DVE so they run in parallel."""
                for g in range(2):
                    masked = (cp == 7 and g == 1)
                    for dsub in range(2):
                        csrc = mp[64 * g:64 * g + 64,
                                  4 * b + dsub:4 * b + dsub + 3:2, :]
                        cdst = qt[g][64 * dsub:64 * dsub + 64, 2 * b:2 * b + 2, :]
                        if masked:
                            qm = qmask[64 * dsub:64 * dsub + 64, 2 * b:2 * b + 2, :]
                            nc.vector.tensor_mul(cdst, csrc, qm)
                        elif g == 0:
                            nc.scalar.copy(cdst, csrc)
                        else:
                            nc.vector.tensor_copy(cdst, csrc)

            for a in range(15):
                cnt = 15 - a
                if a in GP_A:
                    p = prodg.tile([128, 5, 128], BF16, tag="prodg", name="pg")
                    eng = nc.gpsimd
                else:
                    p = prodp.tile([128, 15, 128], BF16, tag="prodp", name="p")
                    eng = nc.vector
                eb = et_all[:, a:a + 1, n0:n0 + 128].to_broadcast((128, cnt, 128))
                if a % 2 == 0:
                    rsrc = rt_sh[:, 0:cnt, n0 + a:n0 + a + 128]
                else:
                    rsrc = rt_all[:, 0:cnt, n0 + a + 1:n0 + a + 129]
                eng.tensor_mul(p[:, 0:cnt, :], eb, rsrc)
                for b in range(4):
                    d_lo = max(a + 1, 4 * b)
                    d_hi = 4 * b + 4
                    if d_lo >= d_hi:
                        continue
                    nc.tensor.matmul(mp[:, d_lo:d_hi, :], ident[:],
                                     p[:, d_lo - a - 1:d_hi - a - 1, :],
                                     start=(a == 0), stop=(a == LAST_A[b]))
                # per-bank evacuation right after the closing matmul
                if a == 2:
                    qt_copy(0)
                elif a == 6:
                    qt_copy(1)
                elif a == 10:
                    qt_copy(2)
                elif a == 14:
                    qt_copy(3)

            for g in range(2):
                cb = 8 * g + cp
                osb = osbp.tile([128, 1024], BF16, tag="osbp", name="osb")
                for h in range(2):
                    wp = wps.tile([128, 512], F32, tag="wps", name="wp")
                    for k in range(8):
                        nc.tensor.matmul(wp[:], qt[g][:, k, :],
                                         ww_sb[:, k, h * 512:(h + 1) * 512],
                                         start=(k == 0), stop=(k == 7))
                    if g == 0:
                        nc.scalar.copy(osb[:, h * 512:(h + 1) * 512], wp[:])
                    else:
                        nc.vector.tensor_copy(osb[:, h * 512:(h + 1) * 512], wp[:])
                nc.gpsimd.dma_start(out_d[cb], osb[:])

        # ---------------- interleaved program ----------------
        reader(0)
        rtsh(0, 0, 511)                      # needs jj0 R
        reader(2)
        # prefetch remaining x windows (WAR on jj0/jj2 matmuls via Tile)
        xk[1] = xkp.tile([128, 8, 512], BF16, tag="xk", name="xk1")
        nc.gpsimd.dma_start(xk[1][:], xw_d[1].rearrange("k p c -> p k c"))
        xk[3] = xkp.tile([128, 8, 512], BF16, tag="xk", name="xk3")
        nc.gpsimd.dma_start(xk[3][:], xw_d[3].rearrange("k p c -> p k c"))
        softz(0)                             # needs jj0+jj2 L
        etmul(0)
        rtsh(1, 0, 160)                      # needs jj2 R -> unblocks cp0
        chunk(0)
        rtsh(1, 160, 511)
        rtsh(0, 1023, 1039)                  # jj2 g0 tail
        etmul(1)
        chunk(1)
        etmul(2)
        chunk(2)
        etmul(3)
        reader(1)
        rtsh(0, 511, 1023)                   # needs jj1 R
        reader(3)
        rtsh(1, 511, 544)                    # needs jj3 R -> unblocks cp3
        softz(1)                             # needs jj1+jj3 L
        etmul(4)
        chunk(3)
        reader(4)
        rtsh(1, 544, 1023)
        rtsh(1, 1023, 1039)                  # needs jj4 halo R
        etmul(5)
        etmul(6)
        etmul(7)
        chunk(4)
        chunk(5)
        chunk(6)
        chunk(7)

    nc.compile()
    return nc


def _host_prep(x, W_r, b_r, W_w, b_w):
    """Build the 8 per-core input maps."""
    xf = np.asarray(x, np.float32).reshape(B * N, PD)
    wr = np.asarray(W_r, np.float32).astype(bfnp)
    ww = np.asarray(W_w, np.float32).astype(bfnp)
    br = np.ascontiguousarray(
        np.asarray(b_r, np.float32).reshape(16, 128).T)
    wr_t = np.ascontiguousarray(wr.reshape(8, 128, 2048))
    ww_t = np.ascontiguousarray(ww.reshape(8, 128, 1024))
    ident = np.eye(128, dtype=np.float32).astype(bfnp)

    in_maps = []
    for c in range(8):
        lo = c * ROWS_PER_CORE
        chunk = np.zeros((ROWS_PER_CORE + HALO, PD), np.float32)
        chunk[:ROWS_PER_CORE] = xf[lo:lo + ROWS_PER_CORE]
        if c % 2 == 0:
            chunk[ROWS_PER_CORE:] = xf[lo + ROWS_PER_CORE: lo + ROWS_PER_CORE + HALO]
        cb = chunk.astype(bfnp)
        # xw[w, ks, k, n] = chunk[512*w + n, 128*ks + k]
        xw = np.ascontiguousarray(
            cb[:ROWS_PER_CORE].reshape(4, 512, 8, 128).transpose(0, 2, 3, 1))
        xh = np.ascontiguousarray(
            cb[ROWS_PER_CORE:].reshape(HALO, 8, 128).transpose(1, 2, 0))
        qmask = np.ones((128, 8, 128), np.float32)
        if c % 2 == 1:
            dsub = (np.arange(128)[:, None, None] // 64)
            k = np.arange(8)[None, :, None]
            n = np.arange(128)[None, None, :]
            bad = (n >= 112) & ((n - 112 + 2 * k + dsub) >= 16)
            qmask[np.broadcast_to(bad, (128, 8, 128))] = 0.0
        in_maps.append({
            "xw": xw, "xh": xh,
            "wr": wr_t, "ww": ww_t, "br": br,
            "ident": ident, "qmask": qmask.astype(bfnp),
        })
    return in_maps


def kernel(x, W_r, b_r, W_w, b_w):
    if "nc" not in _cache:
        _cache["nc"] = _build()
    nc = _cache["nc"]
    in_maps = _host_prep(x, W_r, b_r, W_w, b_w)
    res = bass_utils.run_bass_kernel_spmd(nc, in_maps, core_ids=list(range(8)))
    out = np.concatenate([np.asarray(r["out"], np.float32)
                          .reshape(ROWS_PER_CORE, ALL)
                          for r in res.results], axis=0)
    out = out.reshape(B, N, ALL)
    out += np.asarray(b_w, np.float32).reshape(1, 1, ALL)
    return np.ascontiguousarray(out)
